# revision 1
# baseline (speedup 1.0000x reference)
"""Causal self-attention (B=2, T=2048, C=2048, H=16, D=128) on 8 trn2 cores.

Sharding: tensor-parallel over heads x data-parallel over batch.
Core c handles batch c//4, heads [4*(c%4) .. 4*(c%4)+4). Each core computes
qkv projection for its 4 heads, RoPE, causal attention, and a partial
output projection (its heads' rows of W_proj); the host sums the 4 partials
per batch.

Kernel structure (per core), all matmuls in fp32r (full PE rate @ N=512):
  Phase 1: QKV projection.
    Q,K produced transposed  (Q^T[d,t] = W_q^T x^T)  -> RoPE fused into the
    PSUM evacuation on DVE -> spilled to DRAM scratch.
    V produced natural      (V[t,d]   = x W_v)       -> DRAM scratch.
    All big DMAs split per k-tile so matmuls start as chunks land.
  Phase 2: attention per head, S^T orientation:
    S^T[k,q] = K^T.T @ Q^T   (one 128x512 matmul per tile, no contraction loop)
    causal mask on diagonal tiles = additive -1e30 on PSUM pre-exp (DVE)
    P^T = exp(S^T * 1/sqrt(D))  on ACT (no max subtraction -- scores are O(5))
    denominators: ones^T @ P^T accumulated in a [1,512] PSUM bank (PE)
    O^T[d,q] += V.T @ P^T    accumulated in PSUM over k-blocks
    normalize on evacuation: O^T * recip(broadcast(denom))
  Phase 3: partial proj: out[t,c] = sum_h O_h^T.T @ Wp_h   (O^T is already
    the required lhsT layout -- the whole kernel needs zero transposes).
"""

import contextlib
import math
import os

import numpy as np

B, T, C = 2, 2048, 2048
H, D = 16, 128
HPC = 4  # heads per core
NCORES = 8

_CACHE = {}


def _build_program():
    import concourse.tile as tile
    from concourse import bacc, mybir

    f32 = mybir.dt.float32
    f32r = mybir.dt.float32r
    Exp = mybir.ActivationFunctionType.Exp
    SCALE = 1.0 / math.sqrt(float(D))

    nc = bacc.Bacc(
        "TRN2", target_bir_lowering=False, debug=False, num_devices=NCORES
    )

    xT = nc.dram_tensor("xT", [C, T], f32r, kind="ExternalInput").ap()
    wqk = nc.dram_tensor("wqk", [C, 8 * 128], f32r, kind="ExternalInput").ap()
    wv = nc.dram_tensor("wv", [C, HPC * D], f32r, kind="ExternalInput").ap()
    wp = nc.dram_tensor("wp", [HPC * D, C], f32r, kind="ExternalInput").ap()
    onesr = nc.dram_tensor("onesr", [128, 128], f32r, kind="ExternalInput").ap()
    cosT = nc.dram_tensor("cosT", [128, T], f32, kind="ExternalInput").ap()
    sinTs = nc.dram_tensor("sinTs", [128, T], f32, kind="ExternalInput").ap()
    masks = nc.dram_tensor("masks", [4, 128, 512], f32, kind="ExternalInput").ap()
    out = nc.dram_tensor("out", [T, C], f32, kind="ExternalOutput").ap()

    KT = C // 128  # 16 contraction tiles
    NTB = T // 512  # 4 t-blocks

    with tile.TileContext(nc) as tc:
        with (
            tc.tile_pool(name="consts", bufs=1) as consts,
            tc.tile_pool(name="dram", bufs=1, space="DRAM") as dramp,
        ):
            es = contextlib.ExitStack()
            p2stp = es.enter_context(
                tc.tile_pool(name="p2st", bufs=5, space="PSUM")
            )
            cos_sb = consts.tile([128, T], f32, tag="cos")
            nc.sync.dma_start(out=cos_sb, in_=cosT)
            sin_sb = consts.tile([128, T], f32, tag="sin")
            nc.sync.dma_start(out=sin_sb, in_=sinTs)
            ones_sb = consts.tile([128, 128], f32r, tag="ones")
            nc.sync.dma_start(out=ones_sb, in_=onesr)

            # Per-chunk DRAM scratch so phase-2 loads can chase phase-1
            # writes chunk-by-chunk instead of waiting for whole tensors.
            qkt_dram = [
                [
                    dramp.tile([128, 512], f32r, tag=f"qkt{m}_{tb}",
                               name=f"qkt{m}_{tb}")
                    for tb in range(NTB)
                ]
                for m in range(8)
            ]
            vsc_dram = [
                dramp.tile([128, HPC * D], f32r, tag=f"vsc{i}", name=f"vsc{i}")
                for i in range(T // 128)
            ]

            # ---------------- Phase 1: QKV projection ----------------
            with (
                tc.tile_pool(name="p1x", bufs=2) as p1x,
                tc.tile_pool(name="p1w", bufs=1) as p1w,
                tc.tile_pool(name="p1wv", bufs=1) as p1wv,
                tc.tile_pool(name="p1e", bufs=2) as p1e,
                tc.tile_pool(name="p1ps", bufs=2, space="PSUM") as p1ps,
            ):
                # All 8 q/k weight M-tiles resident (64KB/part); chunked per k
                # and interleaved with the first x block so the first matmul
                # chain starts after ~2 chunks instead of the whole preload.
                wqkg = p1w.tile([128, KT, 8, 128], f32r, tag="wqkg")
                wv_sb = p1wv.tile([128, KT, HPC * D], f32r, tag="wv")
                xtb0 = p1x.tile([128, KT, 512], f32r, tag="xtb")
                MORD = (0, 4, 1, 5, 2, 6, 3, 7)

                def load_wm(m):
                    nc.sync.dma_start(
                        out=wqkg[:, :, m, :],
                        in_=wqk[:, m * 128 : (m + 1) * 128].rearrange(
                            "(k p) c -> p k c", p=128
                        ),
                    )

                # First compute chain (m=0) needs just its own weight column
                # and the first x chunks; stream the rest behind it.
                load_wm(MORD[0])
                load_wm(MORD[1])
                for k in range(KT):
                    nc.sync.dma_start(
                        out=xtb0[:, k], in_=xT[k * 128 : (k + 1) * 128, 0:512]
                    )
                    if k % 2 == 0 and k // 2 + 2 < 8:
                        load_wm(MORD[k // 2 + 2])
                for k in range(KT):
                    nc.sync.dma_start(
                        out=wv_sb[:, k], in_=wv[k * 128 : (k + 1) * 128, :]
                    )
                for tb in range(NTB):
                    tsl = slice(tb * 512, (tb + 1) * 512)
                    if tb == 0:
                        xtb = xtb0
                    else:
                        xtb = p1x.tile([128, KT, 512], f32r, tag="xtb",
                                       name="xtb")
                        for k in range(KT):
                            nc.sync.dma_start(
                                out=xtb[:, k],
                                in_=xT[k * 128 : (k + 1) * 128, tsl],
                            )
                    for m in (0, 4, 1, 5, 2, 6, 3, 7):
                        ps = p1ps.tile([128, 512], f32, tag="qk")
                        for k in range(KT):
                            nc.tensor.matmul(
                                ps,
                                lhsT=wqkg[:, k, m, :],
                                rhs=xtb[:, k, :],
                                start=(k == 0),
                                stop=(k == KT - 1),
                            )
                        # RoPE fused with PSUM evacuation.
                        qk_sb = p1e.tile([128, 512], f32r, tag="qke")
                        tmp = p1e.tile([128, 512], f32, tag="rtmp")
                        nc.vector.tensor_mul(
                            tmp[0:64], ps[64:128], sin_sb[0:64, tsl]
                        )
                        nc.vector.tensor_mul(
                            tmp[64:128], ps[0:64], sin_sb[64:128, tsl]
                        )
                        nc.vector.tensor_mul(qk_sb, ps, cos_sb[:, tsl])
                        nc.vector.tensor_add(qk_sb, qk_sb, tmp)
                        nc.scalar.dma_start(out=qkt_dram[m][tb], in_=qk_sb)
                    for tsub in range(4):
                        csl = slice(tsub * 128, (tsub + 1) * 128)
                        psv = p1ps.tile([128, 512], f32, tag="v", bufs=1)
                        for k in range(KT):
                            nc.tensor.matmul(
                                psv,
                                lhsT=xtb[:, k, csl],
                                rhs=wv_sb[:, k, :],
                                start=(k == 0),
                                stop=(k == KT - 1),
                            )
                        v_sb = p1e.tile([128, 512], f32r, tag="ve")
                        nc.scalar.copy(v_sb, psv)
                        nc.scalar.dma_start(
                            out=vsc_dram[tb * 4 + tsub], in_=v_sb
                        )

            # ---------------- Phases 2+3 share the O^T tiles -------------
            with tc.tile_pool(name="o2", bufs=1) as o2p:
                out2T = [
                    o2p.tile([128, T], f32r, tag=f"o2_{h}", name=f"o2_{h}")
                    for h in range(HPC)
                ]
                _phase2(tc, nc, f32, f32r, Exp, SCALE, KT, qkt_dram,
                        vsc_dram, masks, ones_sb, out2T, p2stp)
                es.close()
                _phase3(tc, nc, f32, f32r, out2T, wp, out)
    nc.compile()
    return nc


def _phase2(tc, nc, f32, f32r, Exp, SCALE, KT, qkt_dram, vsc_dram, masks,
            ones_sb, out2T, p2stp):
    with (
        tc.tile_pool(name="p2m", bufs=1) as p2m,
        tc.tile_pool(name="p2qkv", bufs=2) as p2qkv,
        tc.tile_pool(name="p2pt", bufs=3) as p2pt,
        tc.tile_pool(name="p2s", bufs=2) as p2s,
        tc.tile_pool(name="p2pv", bufs=2, space="PSUM") as p2pv,
        tc.tile_pool(name="p2dn", bufs=1, space="PSUM") as p2dn,
    ):
        mask_sb = p2m.tile([128, 4, 512], f32, tag="mask")
        nc.sync.dma_start(out=mask_sb, in_=masks.rearrange("j p q -> p j q"))
        for h in range(HPC):
            qt = p2qkv.tile([128, T], f32r, tag="qt")
            kt = p2qkv.tile([128, T], f32r, tag="kt")
            vt = p2qkv.tile([128, KT, 128], f32r, tag="vt")
            for tb in range(4):
                s = slice(tb * 512, (tb + 1) * 512)
                nc.sync.dma_start(out=kt[:, s], in_=qkt_dram[4 + h][tb])
                nc.sync.dma_start(out=qt[:, s], in_=qkt_dram[h][tb])
                for tsub in range(4):
                    i = tb * 4 + tsub
                    nc.sync.dma_start(
                        out=vt[:, i],
                        in_=vsc_dram[i][:, h * 128 : (h + 1) * 128],
                    )
            for qb in range(4):  # ascending: chases phase-1 output chunks
                qsl = slice(qb * 512, (qb + 1) * 512)
                pv = p2pv.tile([128, 512], f32, tag="pv")
                dn = p2dn.tile([128, 512], f32, tag="dn")
                nk = 4 * (qb + 1)
                for kb in range(nk):
                    st = p2stp.tile([128, 512], f32, tag="st")
                    nc.tensor.matmul(
                        st,
                        lhsT=kt[:, kb * 128 : (kb + 1) * 128],
                        rhs=qt[:, qsl],
                        start=True,
                        stop=True,
                    )
                    if kb >= qb * 4:
                        nc.vector.tensor_add(st, st, mask_sb[:, kb - qb * 4, :])
                    pt = p2pt.tile([128, 512], f32r, tag="pt")
                    nc.scalar.activation(pt, st, Exp, scale=SCALE)
                    nc.tensor.matmul(
                        dn,
                        lhsT=ones_sb,
                        rhs=pt,
                        start=(kb == 0),
                        stop=(kb == nk - 1),
                    )
                    nc.tensor.matmul(
                        pv,
                        lhsT=vt[:, kb, :],
                        rhs=pt,
                        start=(kb == 0),
                        stop=(kb == nk - 1),
                    )
                # dn already holds the denominator on every partition
                # (ones[128,128] lhsT): reciprocal + normalize, no broadcast.
                rb2 = p2s.tile([128, 512], f32, tag="rb2")
                nc.vector.reciprocal_approx_fast(out=rb2, in_=dn)
                nc.vector.tensor_mul(out2T[h][:, qsl], pv, rb2)


def _phase3(tc, nc, f32, f32r, out2T, wp, out):
    with (
        tc.tile_pool(name="p3w", bufs=1) as p3w,
        tc.tile_pool(name="p3e", bufs=4) as p3e,
        tc.tile_pool(name="p3ps", bufs=8, space="PSUM") as p3ps,
    ):
        wps = [
            p3w.tile([128, T], f32r, tag=f"wp{i}", name=f"wp{i}")
            for i in range(HPC)
        ]
        for i in range(HPC):
            nc.sync.dma_start(out=wps[i], in_=wp[i * 128 : (i + 1) * 128, :])
        for t in range(T // 128):
            tsl = slice(t * 128, (t + 1) * 128)
            pos = [
                p3ps.tile([128, 512], f32, tag="po", name=f"po{t}_{cb}")
                for cb in range(4)
            ]
            # hd outer / cb inner: 4 matmuls share one LDWEIGHTS.
            for hd in range(HPC):
                for cb in range(4):
                    nc.tensor.matmul(
                        pos[cb],
                        lhsT=out2T[hd][:, tsl],
                        rhs=wps[hd][:, cb * 512 : (cb + 1) * 512],
                        start=(hd == 0),
                        stop=(hd == HPC - 1),
                    )
            for cb in range(4):
                ob = p3e.tile([128, 512], f32, tag="ob")
                nc.vector.tensor_copy(ob, pos[cb])
                nc.sync.dma_start(
                    out=out[tsl, cb * 512 : (cb + 1) * 512], in_=ob
                )


def _get_program():
    if "nc" not in _CACHE:
        _CACHE["nc"] = _build_program()
    return _CACHE["nc"]


def make_in_maps(x, cos, sin, W_qkv, W_proj):
    """Host-side sharding: per-core input dicts (numpy, fp32)."""
    x = np.asarray(x, dtype=np.float32)
    cos = np.asarray(cos, dtype=np.float32)
    sin = np.asarray(sin, dtype=np.float32)
    W_qkv = np.asarray(W_qkv, dtype=np.float32)
    W_proj = np.asarray(W_proj, dtype=np.float32)

    cosT = np.ascontiguousarray(np.tile(cos.T, (2, 1)))  # [128, T]
    sinT = np.ascontiguousarray(np.concatenate([-sin.T, sin.T], axis=0))
    q_idx = np.arange(512)[None, None, :]
    k_idx = np.arange(128)[None, :, None]
    j_idx = np.arange(4)[:, None, None]
    masks = np.where(
        q_idx >= j_idx * 128 + k_idx, 0.0, -1.0e30
    ).astype(np.float32)  # [4, 128, 512] additive
    onesr = np.ones((128, 128), dtype=np.float32)

    in_maps = []
    for core in range(NCORES):
        b, hg = core // 4, core % 4
        csl = slice(hg * 512, (hg + 1) * 512)
        wqk_np = np.ascontiguousarray(
            np.concatenate(
                [W_qkv[:, csl], W_qkv[:, C + hg * 512 : C + (hg + 1) * 512]],
                axis=1,
            )
        )
        wv_np = np.ascontiguousarray(
            W_qkv[:, 2 * C + hg * 512 : 2 * C + (hg + 1) * 512]
        )
        wp_np = np.ascontiguousarray(W_proj[hg * 512 : (hg + 1) * 512, :])
        xT_np = np.ascontiguousarray(x[b].T)
        in_maps.append(
            {
                "xT": xT_np,
                "wqk": wqk_np,
                "wv": wv_np,
                "wp": wp_np,
                "onesr": onesr,
                "cosT": cosT,
                "sinTs": sinT,
                "masks": masks,
            }
        )
    return in_maps


def kernel(x, cos, sin, W_qkv, W_proj):
    from concourse.bass_utils import run_bass_kernel_spmd

    nc = _get_program()
    in_maps = make_in_maps(x, cos, sin, W_qkv, W_proj)
    trace = bool(int(os.environ.get("KERNEL_TRACE", "0")))
    res = run_bass_kernel_spmd(
        nc, in_maps, core_ids=list(range(NCORES)), trace=trace
    )
    if trace:
        _CACHE["last_results"] = res
        if res.exec_time_ns is not None:
            print(f"HW exec time: {res.exec_time_ns} ns")

    out = np.zeros((B, T, C), dtype=np.float32)
    for core in range(NCORES):
        out[core // 4] += res.results[core]["out"]
    return out



# revision 2
# speedup vs baseline: 1.2085x; 1.2085x over previous
"""Causal self-attention (B=2, T=2048, C=2048, H=16, D=128) on 8 trn2 cores.

Sharding: tensor-parallel over heads x data-parallel over batch.
Core c handles batch c//4, heads [4*(c%4) .. 4*(c%4)+4). Each core computes
qkv projection for its 4 heads, RoPE, causal attention, and a partial
output projection (its heads' rows of W_proj); the host sums the 4 partials
per batch.

v2 design (all matmuls bf16 -> PSUM f32; ~4e-3 max-rel error):
  - Q^T/K^T/V live in SBUF in bf16 for the whole kernel: no DRAM scratch
    round-trip between projection and attention.
  - Phase 1: QKV projection streamed per 512-col t-block; Q,K produced
    transposed (W^T x^T) with RoPE fused into the PSUM evacuation on DVE;
    V natural via x-as-stationary, evacuated to SBUF by ACT.
  - Phase 2 per (q-block, head), S^T orientation (zero transposes):
      S^T[k,q] = K^T.T @ Q^T; diagonal tiles only compute the valid column
      range (q_rel >= j*128), trimming ~13% of S/exp/PV/dn work.
      P^T = exp(S^T/sqrt(D)) on ACT; causal masking is a multiplicative
      0/1 upper-triangular [128,128] mask on DVE applied post-exp to the
      single mixed tile (cheaper than additive [128,512] masks pre-exp).
      denominators: ones^T @ P^T accumulated on PE; O^T = PV / denom.
  - Phase 3 interleaved per q-block (out DMA streams during attention):
    out[t,c] = sum_h O_h^T.T @ Wp_h; PSUM evacuated by ACT (DMA can't
    read PSUM), DMA'd as f32 partials summed on host.
"""

import math
import os

import numpy as np

B, T, C = 2, 2048, 2048
H, D = 16, 128
HPC = 4  # heads per core
NCORES = 8

_CACHE = {}


def _build_program():
    import concourse.tile as tile
    from concourse import bacc, mybir

    f32 = mybir.dt.float32
    bf16 = mybir.dt.bfloat16
    Exp = mybir.ActivationFunctionType.Exp
    SCALE = 1.0 / math.sqrt(float(D))

    nc = bacc.Bacc(
        "TRN2", target_bir_lowering=False, debug=False, num_devices=NCORES
    )

    xT = nc.dram_tensor("xT", [C, T], bf16, kind="ExternalInput").ap()
    wqk = nc.dram_tensor("wqk", [C, 8 * 128], bf16, kind="ExternalInput").ap()
    wv = nc.dram_tensor("wv", [C, HPC * D], bf16, kind="ExternalInput").ap()
    wp = nc.dram_tensor("wp", [HPC * D, C], bf16, kind="ExternalInput").ap()
    onesr = nc.dram_tensor("onesr", [128, 128], bf16, kind="ExternalInput").ap()
    cosT = nc.dram_tensor("cosT", [128, T], f32, kind="ExternalInput").ap()
    sinTs = nc.dram_tensor("sinTs", [128, T], f32, kind="ExternalInput").ap()
    mask01 = nc.dram_tensor("mask01", [128, 128], bf16, kind="ExternalInput").ap()
    out = nc.dram_tensor("out", [T, C], f32, kind="ExternalOutput").ap()

    KT = C // 128  # 16 contraction tiles
    NTB = T // 512  # 4 t-blocks
    MORD = (0, 4, 1, 5, 2, 6, 3, 7)

    with tile.TileContext(nc) as tc:
        with (
            tc.tile_pool(name="consts", bufs=1) as consts,
            tc.tile_pool(name="pers", bufs=1) as pers,
        ):
            ones_sb = consts.tile([128, 128], bf16, tag="ones")
            m01_sb = consts.tile([128, 128], bf16, tag="m01")
            cos_sb = consts.tile([128, T], f32, tag="cos")
            sin_sb = consts.tile([128, T], f32, tag="sin")

            qt = [
                pers.tile([128, T], bf16, tag=f"qt{h}", name=f"qt{h}")
                for h in range(HPC)
            ]
            kt = [
                pers.tile([128, T], bf16, tag=f"kt{h}", name=f"kt{h}")
                for h in range(HPC)
            ]
            vt = pers.tile([128, KT, 512], bf16, tag="vt")
            o2 = [
                pers.tile([128, T], bf16, tag=f"o2_{h}", name=f"o2_{h}")
                for h in range(HPC)
            ]
            wps = [
                pers.tile([128, T], bf16, tag=f"wp{i}", name=f"wp{i}")
                for i in range(HPC)
            ]

            # ---------------- Phase 1: QKV projection ----------------
            with (
                tc.tile_pool(name="p1x", bufs=2) as p1x,
                tc.tile_pool(name="p1w", bufs=1) as p1w,
                tc.tile_pool(name="p1e", bufs=2) as p1e,
                tc.tile_pool(name="p1qk", bufs=3, space="PSUM") as p1qk,
                tc.tile_pool(name="p1v", bufs=2, space="PSUM") as p1v,
            ):
                wqkg = p1w.tile([128, 8, KT, 128], bf16, tag="wqkg")
                wv_sb = p1w.tile([128, KT, 512], bf16, tag="wv")

                def load_wm(m):
                    nc.sync.dma_start(
                        out=wqkg[:, m],
                        in_=wqk[:, m * 128 : (m + 1) * 128].rearrange(
                            "(k p) c -> p k c", p=128
                        ),
                    )

                def load_x(xtile, tb):
                    tsl = slice(tb * 512, (tb + 1) * 512)
                    for k in range(KT):
                        nc.sync.dma_start(
                            out=xtile[:, k], in_=xT[k * 128 : (k + 1) * 128, tsl]
                        )

                # Preamble: what the first m-chain needs goes first.
                nc.sync.dma_start(out=ones_sb, in_=onesr)
                nc.sync.dma_start(out=m01_sb, in_=mask01)
                nc.sync.dma_start(out=cos_sb[:, 0:512], in_=cosT[:, 0:512])
                nc.sync.dma_start(out=sin_sb[:, 0:512], in_=sinTs[:, 0:512])
                load_wm(MORD[0])
                xtb0 = p1x.tile([128, KT, 512], bf16, tag="xtb")
                for k in range(KT):
                    nc.sync.dma_start(
                        out=xtb0[:, k], in_=xT[k * 128 : (k + 1) * 128, 0:512]
                    )
                    if k % 2 == 1 and k // 2 + 1 < 8:
                        load_wm(MORD[k // 2 + 1])
                for k in range(KT):
                    nc.sync.dma_start(out=wv_sb[:, k], in_=wv[k * 128 : (k + 1) * 128, :])
                xtb1 = p1x.tile([128, KT, 512], bf16, tag="xtb", name="xtb1")
                load_x(xtb1, 1)
                for tbb in range(1, NTB):
                    s = slice(tbb * 512, (tbb + 1) * 512)
                    nc.sync.dma_start(out=cos_sb[:, s], in_=cosT[:, s])
                    nc.sync.dma_start(out=sin_sb[:, s], in_=sinTs[:, s])
                for i in range(HPC):
                    nc.sync.dma_start(out=wps[i], in_=wp[i * 128 : (i + 1) * 128, :])

                xtbs = [xtb0, xtb1, None, None]
                for tb in range(NTB):
                    tsl = slice(tb * 512, (tb + 1) * 512)
                    if tb + 2 < NTB:
                        xn = p1x.tile([128, KT, 512], bf16, tag="xtb",
                                      name=f"xtb{tb + 2}")
                        load_x(xn, tb + 2)
                        xtbs[tb + 2] = xn
                    xtb = xtbs[tb]
                    for m in MORD:
                        ps = p1qk.tile([128, 512], f32, tag="qk")
                        for k in range(KT):
                            nc.tensor.matmul(
                                ps,
                                lhsT=wqkg[:, m, k, :],
                                rhs=xtb[:, k, :],
                                start=(k == 0),
                                stop=(k == KT - 1),
                            )
                        # RoPE fused with PSUM evacuation (DVE), bf16 out.
                        dst = qt[m][:, tsl] if m < 4 else kt[m - 4][:, tsl]
                        tmp = p1e.tile([128, 512], f32, tag="rtmp")
                        nc.vector.tensor_mul(
                            tmp[0:64], ps[64:128], sin_sb[0:64, tsl]
                        )
                        nc.vector.tensor_mul(
                            tmp[64:128], ps[0:64], sin_sb[64:128, tsl]
                        )
                        tmp2 = p1e.tile([128, 512], f32, tag="rtmp2")
                        nc.vector.tensor_mul(tmp2, ps, cos_sb[:, tsl])
                        nc.vector.tensor_add(dst, tmp2, tmp)
                    for tsub in range(4):
                        psv = p1v.tile([128, 512], f32, tag="v")
                        for k in range(KT):
                            nc.tensor.matmul(
                                psv,
                                lhsT=xtb[:, k, tsub * 128 : (tsub + 1) * 128],
                                rhs=wv_sb[:, k, :],
                                start=(k == 0),
                                stop=(k == KT - 1),
                            )
                        nc.scalar.copy(vt[:, tb * 4 + tsub, :], psv)

            # ------------- Phases 2+3 fused per q-block -------------
            with (
                tc.tile_pool(name="p2st", bufs=3, space="PSUM") as p2stp,
                tc.tile_pool(name="p2pv", bufs=2, space="PSUM") as p2pv,
                tc.tile_pool(name="p2dn", bufs=1, space="PSUM") as p2dn,
                tc.tile_pool(name="p3ps", bufs=2, space="PSUM") as p3ps,
                tc.tile_pool(name="p2pt", bufs=4) as p2pt,
                tc.tile_pool(name="p2s", bufs=2) as p2s,
                tc.tile_pool(name="p3e", bufs=4) as p3e,
            ):
                for qb in range(NTB):
                    qsl = slice(qb * 512, (qb + 1) * 512)
                    nk = 4 * (qb + 1)
                    for h in range(HPC):
                        pv = p2pv.tile([128, 512], f32, tag="pv")
                        dn = p2dn.tile([128, 512], f32, tag="dn")
                        for kb in range(nk):
                            j = kb - 4 * qb  # >=0 on the diagonal group
                            off = j * 128 if j > 0 else 0
                            st = p2stp.tile([128, 512], f32, tag="st")
                            nc.tensor.matmul(
                                st[:, off:],
                                lhsT=kt[h][:, kb * 128 : (kb + 1) * 128],
                                rhs=qt[h][:, qb * 512 + off : (qb + 1) * 512],
                                start=True,
                                stop=True,
                            )
                            pt = p2pt.tile([128, 512], bf16, tag="pt")
                            nc.scalar.activation(
                                pt[:, off:], st[:, off:], Exp, scale=SCALE
                            )
                            if j >= 0:
                                nc.vector.tensor_mul(
                                    pt[:, off : off + 128],
                                    pt[:, off : off + 128],
                                    m01_sb,
                                )
                            nc.tensor.matmul(
                                dn[:, off:],
                                lhsT=ones_sb,
                                rhs=pt[:, off:],
                                start=(kb == 0),
                                stop=(kb == nk - 1),
                            )
                            nc.tensor.matmul(
                                pv[:, off:],
                                lhsT=vt[:, kb, h * 128 : (h + 1) * 128],
                                rhs=pt[:, off:],
                                start=(kb == 0),
                                stop=(kb == nk - 1),
                            )
                        # dn holds the denominator on every partition
                        # (ones[128,128] lhsT): reciprocal + normalize.
                        rb = p2s.tile([128, 512], f32, tag="rb")
                        nc.vector.reciprocal_approx_fast(out=rb, in_=dn)
                        nc.vector.tensor_mul(o2[h][:, qsl], pv, rb)
                    # Phase 3 for this q-block's four 128-row t-tiles.
                    for tt in range(4):
                        t = qb * 4 + tt
                        tsl = slice(t * 128, (t + 1) * 128)
                        for half in range(2):
                            pos = [
                                p3ps.tile([128, 512], f32, tag="po",
                                          name=f"po{t}_{half}_{i}")
                                for i in range(2)
                            ]
                            for hd in range(HPC):
                                for i in range(2):
                                    cb = half * 2 + i
                                    nc.tensor.matmul(
                                        pos[i],
                                        lhsT=o2[hd][:, tsl],
                                        rhs=wps[hd][:, cb * 512 : (cb + 1) * 512],
                                        start=(hd == 0),
                                        stop=(hd == HPC - 1),
                                    )
                            for i in range(2):
                                cb = half * 2 + i
                                ob = p3e.tile([128, 512], f32, tag="ob")
                                nc.scalar.copy(ob, pos[i])
                                nc.sync.dma_start(
                                    out=out[tsl, cb * 512 : (cb + 1) * 512],
                                    in_=ob,
                                )
    nc.compile()
    return nc


def _get_program():
    if "nc" not in _CACHE:
        _CACHE["nc"] = _build_program()
    return _CACHE["nc"]


def make_in_maps(x, cos, sin, W_qkv, W_proj):
    """Host-side sharding: per-core input dicts (bf16 streams)."""
    import ml_dtypes

    bf = ml_dtypes.bfloat16
    x = np.asarray(x, dtype=np.float32)
    cos = np.asarray(cos, dtype=np.float32)
    sin = np.asarray(sin, dtype=np.float32)
    W_qkv = np.asarray(W_qkv, dtype=np.float32)
    W_proj = np.asarray(W_proj, dtype=np.float32)

    cosT = np.ascontiguousarray(np.tile(cos.T, (2, 1)))  # [128, T]
    sinT = np.ascontiguousarray(np.concatenate([-sin.T, sin.T], axis=0))
    onesr = np.ones((128, 128), dtype=bf)
    # Valid (unmasked) iff q-col >= k-partition within the mixed tile.
    mask01 = np.triu(np.ones((128, 128), dtype=np.float32)).astype(bf)

    in_maps = []
    for core in range(NCORES):
        b, hg = core // 4, core % 4
        csl = slice(hg * 512, (hg + 1) * 512)
        wqk_np = np.ascontiguousarray(
            np.concatenate(
                [W_qkv[:, csl], W_qkv[:, C + hg * 512 : C + (hg + 1) * 512]],
                axis=1,
            )
        ).astype(bf)
        wv_np = np.ascontiguousarray(
            W_qkv[:, 2 * C + hg * 512 : 2 * C + (hg + 1) * 512]
        ).astype(bf)
        wp_np = np.ascontiguousarray(W_proj[hg * 512 : (hg + 1) * 512, :]).astype(bf)
        xT_np = np.ascontiguousarray(x[b].T).astype(bf)
        in_maps.append(
            {
                "xT": xT_np,
                "wqk": wqk_np,
                "wv": wv_np,
                "wp": wp_np,
                "onesr": onesr,
                "cosT": cosT,
                "sinTs": sinT,
                "mask01": mask01,
            }
        )
    return in_maps


def kernel(x, cos, sin, W_qkv, W_proj):
    from concourse.bass_utils import run_bass_kernel_spmd

    nc = _get_program()
    in_maps = make_in_maps(x, cos, sin, W_qkv, W_proj)
    trace = bool(int(os.environ.get("KERNEL_TRACE", "0")))
    res = run_bass_kernel_spmd(
        nc, in_maps, core_ids=list(range(NCORES)), trace=trace
    )
    if trace:
        _CACHE["last_results"] = res
        if res.exec_time_ns is not None:
            print(f"HW exec time: {res.exec_time_ns} ns")

    out = np.zeros((B, T, C), dtype=np.float32)
    for core in range(NCORES):
        out[core // 4] += res.results[core]["out"]
    return out


# revision 3
# speedup vs baseline: 1.2448x; 1.0300x over previous
"""Causal self-attention (B=2, T=2048, C=2048, H=16, D=128) on 8 trn2 cores.

Sharding: tensor-parallel over heads x data-parallel over batch.
Core c handles batch c//4, heads [4*(c%4) .. 4*(c%4)+4). Each core computes
qkv projection for its 4 heads, RoPE, causal attention, and a partial
output projection (its heads' rows of W_proj); the host sums the 4 partials
per batch.

v3 design (all matmuls bf16 -> PSUM f32; ~4e-3 max-rel error):
  - Q^T/K^T/V live in SBUF in bf16 for the whole kernel: no DRAM scratch.
  - Weights are repacked host-side to partition-major so every DMA moves
    contiguous 4KB runs per partition.
  - Phase 1: QKV projection per 512-col t-block; Q,K produced transposed
    (W^T x^T) with RoPE fused into the PSUM evacuation on DVE; V natural
    via x-as-stationary, evacuated to SBUF by ACT.
  - Phase 2 per (q-block, head), S^T orientation, software-pipelined
    depth 3 (S matmuls run ahead of dn/pv so the ACT exp latency is
    hidden). Diagonal tiles only compute the valid column range
    (q_rel >= j*128); causal masking is a multiplicative 0/1 triangular
    [128,128] bf16 mask on DVE applied post-exp to the one mixed tile.
    denominators: ones^T @ P^T on PE; O^T = PV * recip(dn).
  - Phase 3 interleaved per q-block; PSUM evacuated by DVE (ACT keeps
    doing exp; DMA can't read PSUM); out f32 partials summed on host.
  - PSUM banks: st ring 4 (shared with phase-3 po tiles) + pv 2 + dn 2.
"""

import math
import os

import numpy as np

B, T, C = 2, 2048, 2048
H, D = 16, 128
HPC = 4  # heads per core
NCORES = 8

_CACHE = {}


def _build_program():
    import concourse.tile as tile
    from concourse import bacc, mybir

    f32 = mybir.dt.float32
    bf16 = mybir.dt.bfloat16
    Exp = mybir.ActivationFunctionType.Exp
    SCALE = 1.0 / math.sqrt(float(D))

    nc = bacc.Bacc(
        "TRN2", target_bir_lowering=False, debug=False, num_devices=NCORES
    )

    KT = C // 128  # 16 contraction tiles
    NTB = T // 512  # 4 t-blocks
    MORD = (0, 4, 1, 5, 2, 6, 3, 7)

    # Partition-major packed layouts (see make_in_maps).
    xP = nc.dram_tensor("xP", [128, NTB, KT, 512], bf16, kind="ExternalInput").ap()
    wqkP = nc.dram_tensor("wqkP", [128, 8 * KT * 128], bf16, kind="ExternalInput").ap()
    wvP = nc.dram_tensor("wvP", [128, KT * 512], bf16, kind="ExternalInput").ap()
    wp = nc.dram_tensor("wp", [HPC * D, C], bf16, kind="ExternalInput").ap()
    onesr = nc.dram_tensor("onesr", [128, 128], bf16, kind="ExternalInput").ap()
    cosT = nc.dram_tensor("cosT", [128, T], f32, kind="ExternalInput").ap()
    sinTs = nc.dram_tensor("sinTs", [128, T], f32, kind="ExternalInput").ap()
    mask01 = nc.dram_tensor("mask01", [128, 128], bf16, kind="ExternalInput").ap()
    out = nc.dram_tensor("out", [T, C], f32, kind="ExternalOutput").ap()

    with tile.TileContext(nc) as tc:
        with (
            tc.tile_pool(name="consts", bufs=1) as consts,
            tc.tile_pool(name="pers", bufs=1) as pers,
        ):
            ones_sb = consts.tile([128, 128], bf16, tag="ones")
            m01_sb = consts.tile([128, 128], bf16, tag="m01")
            cos_sb = consts.tile([128, T], f32, tag="cos")
            sin_sb = consts.tile([128, T], f32, tag="sin")

            qt = [
                pers.tile([128, T], bf16, tag=f"qt{h}", name=f"qt{h}")
                for h in range(HPC)
            ]
            kt = [
                pers.tile([128, T], bf16, tag=f"kt{h}", name=f"kt{h}")
                for h in range(HPC)
            ]
            vt = pers.tile([128, KT, 512], bf16, tag="vt")
            o2 = [
                pers.tile([128, T], bf16, tag=f"o2_{h}", name=f"o2_{h}")
                for h in range(HPC)
            ]
            wps = [
                pers.tile([128, T], bf16, tag=f"wp{i}", name=f"wp{i}")
                for i in range(HPC)
            ]

            # ---------------- Phase 1: QKV projection ----------------
            with (
                tc.tile_pool(name="p1x", bufs=2) as p1x,
                tc.tile_pool(name="p1w", bufs=1) as p1w,
                tc.tile_pool(name="p1e", bufs=2) as p1e,
                tc.tile_pool(name="p1qk", bufs=3, space="PSUM") as p1qk,
                tc.tile_pool(name="p1v", bufs=2, space="PSUM") as p1v,
            ):
                wqkg = p1w.tile([128, 8, KT, 128], bf16, tag="wqkg")
                wv_sb = p1w.tile([128, KT, 512], bf16, tag="wv")

                def load_wm(m):
                    nc.sync.dma_start(
                        out=wqkg[:, m], in_=wqkP[:, m * 2048 : (m + 1) * 2048]
                    )

                def load_x(xtile, tb):
                    for kg in range(4):
                        nc.sync.dma_start(
                            out=xtile[:, kg * 4 : (kg + 1) * 4, :],
                            in_=xP[:, tb, kg * 4 : (kg + 1) * 4, :],
                        )

                # Preamble: what the first m-chain needs goes first.
                nc.sync.dma_start(out=ones_sb, in_=onesr)
                nc.sync.dma_start(out=m01_sb, in_=mask01)
                nc.sync.dma_start(out=cos_sb[:, 0:512], in_=cosT[:, 0:512])
                nc.sync.dma_start(out=sin_sb[:, 0:512], in_=sinTs[:, 0:512])
                load_wm(MORD[0])
                xtb0 = p1x.tile([128, KT, 512], bf16, tag="xtb")
                load_x(xtb0, 0)
                for m in MORD[1:]:
                    load_wm(m)
                for kg in range(4):
                    nc.sync.dma_start(
                        out=wv_sb[:, kg * 4 : (kg + 1) * 4, :],
                        in_=wvP[:, kg * 2048 : (kg + 1) * 2048],
                    )
                xtb1 = p1x.tile([128, KT, 512], bf16, tag="xtb", name="xtb1")
                load_x(xtb1, 1)
                for tbb in range(1, NTB):
                    s = slice(tbb * 512, (tbb + 1) * 512)
                    nc.sync.dma_start(out=cos_sb[:, s], in_=cosT[:, s])
                    nc.sync.dma_start(out=sin_sb[:, s], in_=sinTs[:, s])
                for i in range(HPC):
                    nc.sync.dma_start(out=wps[i], in_=wp[i * 128 : (i + 1) * 128, :])

                xtbs = [xtb0, xtb1, None, None]
                for tb in range(NTB):
                    tsl = slice(tb * 512, (tb + 1) * 512)
                    if tb + 2 < NTB:
                        xn = p1x.tile([128, KT, 512], bf16, tag="xtb",
                                      name=f"xtb{tb + 2}")
                        load_x(xn, tb + 2)
                        xtbs[tb + 2] = xn
                    xtb = xtbs[tb]
                    for m in MORD:
                        ps = p1qk.tile([128, 512], f32, tag="qk")
                        for k in range(KT):
                            nc.tensor.matmul(
                                ps,
                                lhsT=wqkg[:, m, k, :],
                                rhs=xtb[:, k, :],
                                start=(k == 0),
                                stop=(k == KT - 1),
                            )
                        # RoPE fused with PSUM evacuation (DVE), bf16 out.
                        dst = qt[m][:, tsl] if m < 4 else kt[m - 4][:, tsl]
                        tmp = p1e.tile([128, 512], f32, tag="rtmp")
                        nc.vector.tensor_mul(
                            tmp[0:64], ps[64:128], sin_sb[0:64, tsl]
                        )
                        nc.vector.tensor_mul(
                            tmp[64:128], ps[0:64], sin_sb[64:128, tsl]
                        )
                        tmp2 = p1e.tile([128, 512], f32, tag="rtmp2")
                        nc.vector.tensor_mul(tmp2, ps, cos_sb[:, tsl])
                        nc.vector.tensor_add(dst, tmp2, tmp)
                    for tsub in range(4):
                        psv = p1v.tile([128, 512], f32, tag="v")
                        for k in range(KT):
                            nc.tensor.matmul(
                                psv,
                                lhsT=xtb[:, k, tsub * 128 : (tsub + 1) * 128],
                                rhs=wv_sb[:, k, :],
                                start=(k == 0),
                                stop=(k == KT - 1),
                            )
                        nc.scalar.copy(vt[:, tb * 4 + tsub, :], psv)

            # ------------- Phases 2+3 fused per q-block -------------
            with (
                tc.tile_pool(name="p2ps", bufs=1, space="PSUM") as p2ps,
                tc.tile_pool(name="p2pt", bufs=5) as p2pt,
                tc.tile_pool(name="p2s", bufs=2) as p2s,
            ):
                DEPTH = 3
                pending = {}

                def emit_S(qb, h, kb):
                    j = kb - 4 * qb  # >=0 on the diagonal group
                    off = j * 128 if j > 0 else 0
                    st = p2ps.tile([128, 512], f32, tag="st", bufs=4)
                    nc.tensor.matmul(
                        st[:, off:],
                        lhsT=kt[h][:, kb * 128 : (kb + 1) * 128],
                        rhs=qt[h][:, qb * 512 + off : (qb + 1) * 512],
                        start=True,
                        stop=True,
                    )
                    pt = p2pt.tile([128, 512], bf16, tag="pt", bufs=5)
                    nc.scalar.activation(pt[:, off:], st[:, off:], Exp, scale=SCALE)
                    if j >= 0:
                        nc.vector.tensor_mul(
                            pt[:, off : off + 128], pt[:, off : off + 128], m01_sb
                        )
                    pending[(qb, h, kb)] = (pt, off)

                for qb in range(NTB):
                    qsl = slice(qb * 512, (qb + 1) * 512)
                    nk = 4 * (qb + 1)
                    for h in range(HPC):
                        pv = p2ps.tile([128, 512], f32, tag="pv", bufs=2)
                        dn = p2ps.tile([128, 512], f32, tag="dn", bufs=2)
                        for kb in range(min(DEPTH, nk)):
                            if (qb, h, kb) not in pending:
                                emit_S(qb, h, kb)
                        for kb in range(nk):
                            if kb + DEPTH < nk:
                                emit_S(qb, h, kb + DEPTH)
                            pt, off = pending.pop((qb, h, kb))
                            nc.tensor.matmul(
                                dn[:, off:],
                                lhsT=ones_sb,
                                rhs=pt[:, off:],
                                start=(kb == 0),
                                stop=(kb == nk - 1),
                            )
                            nc.tensor.matmul(
                                pv[:, off:],
                                lhsT=vt[:, kb, h * 128 : (h + 1) * 128],
                                rhs=pt[:, off:],
                                start=(kb == 0),
                                stop=(kb == nk - 1),
                            )
                        # dn holds the denominator on every partition.
                        rb = p2s.tile([128, 512], f32, tag="rb")
                        nc.vector.reciprocal_approx_fast(out=rb, in_=dn)
                        nc.vector.tensor_mul(o2[h][:, qsl], pv, rb)
                    # Prime next q-block's first head so its exps overlap
                    # this q-block's projection matmuls below.
                    if qb + 1 < NTB:
                        for kb in range(DEPTH):
                            emit_S(qb + 1, 0, kb)
                    # Phase 3 for this q-block's four 128-row t-tiles.
                    for tt in range(4):
                        t = qb * 4 + tt
                        tsl = slice(t * 128, (t + 1) * 128)
                        for half in range(2):
                            pos = [
                                p2ps.tile([128, 512], f32, tag="st", bufs=4,
                                          name=f"po{t}_{half}_{i}")
                                for i in range(2)
                            ]
                            for hd in range(HPC):
                                for i in range(2):
                                    cb = half * 2 + i
                                    nc.tensor.matmul(
                                        pos[i],
                                        lhsT=o2[hd][:, tsl],
                                        rhs=wps[hd][:, cb * 512 : (cb + 1) * 512],
                                        start=(hd == 0),
                                        stop=(hd == HPC - 1),
                                    )
                            for i in range(2):
                                cb = half * 2 + i
                                ob = p2s.tile([128, 512], f32, tag="ob", bufs=4)
                                nc.vector.tensor_copy(ob, pos[i])
                                nc.sync.dma_start(
                                    out=out[tsl, cb * 512 : (cb + 1) * 512],
                                    in_=ob,
                                )
    nc.compile()
    return nc


def _get_program():
    if "nc" not in _CACHE:
        _CACHE["nc"] = _build_program()
    return _CACHE["nc"]


def make_in_maps(x, cos, sin, W_qkv, W_proj):
    """Host-side sharding: per-core input dicts (bf16, partition-major)."""
    import ml_dtypes

    bf = ml_dtypes.bfloat16
    x = np.asarray(x, dtype=np.float32)
    cos = np.asarray(cos, dtype=np.float32)
    sin = np.asarray(sin, dtype=np.float32)
    W_qkv = np.asarray(W_qkv, dtype=np.float32)
    W_proj = np.asarray(W_proj, dtype=np.float32)

    cosT = np.ascontiguousarray(np.tile(cos.T, (2, 1)))  # [128, T]
    sinT = np.ascontiguousarray(np.concatenate([-sin.T, sin.T], axis=0))
    onesr = np.ones((128, 128), dtype=bf)
    # Valid (unmasked) iff q-col >= k-partition within the mixed tile.
    mask01 = np.triu(np.ones((128, 128), dtype=np.float32)).astype(bf)

    in_maps = []
    for core in range(NCORES):
        b, hg = core // 4, core % 4
        csl = slice(hg * 512, (hg + 1) * 512)
        wqk_np = np.concatenate(
            [W_qkv[:, csl], W_qkv[:, C + hg * 512 : C + (hg + 1) * 512]],
            axis=1,
        ).astype(bf)  # [C, 1024]
        # -> [128, (m, k, 128)] partition-major
        wqkP = np.ascontiguousarray(
            wqk_np.reshape(16, 128, 8, 128).transpose(1, 2, 0, 3).reshape(128, -1)
        )
        wv_np = W_qkv[:, 2 * C + hg * 512 : 2 * C + (hg + 1) * 512].astype(bf)
        wvP = np.ascontiguousarray(
            wv_np.reshape(16, 128, 512).transpose(1, 0, 2).reshape(128, -1)
        )
        wp_np = np.ascontiguousarray(W_proj[hg * 512 : (hg + 1) * 512, :]).astype(bf)
        # x[b].T [C, T] -> [128, (tb, k, 512)] partition-major
        xT_np = x[b].T.astype(bf)  # [C, T]
        xPm = np.ascontiguousarray(
            xT_np.reshape(16, 128, 4, 512).transpose(1, 2, 0, 3)
        )  # [128, 4, 16, 512]
        in_maps.append(
            {
                "xP": xPm,
                "wqkP": wqkP,
                "wvP": wvP,
                "wp": wp_np,
                "onesr": onesr,
                "cosT": cosT,
                "sinTs": sinT,
                "mask01": mask01,
            }
        )
    return in_maps


def kernel(x, cos, sin, W_qkv, W_proj):
    from concourse.bass_utils import run_bass_kernel_spmd

    nc = _get_program()
    in_maps = make_in_maps(x, cos, sin, W_qkv, W_proj)
    trace = bool(int(os.environ.get("KERNEL_TRACE", "0")))
    res = run_bass_kernel_spmd(
        nc, in_maps, core_ids=list(range(NCORES)), trace=trace
    )
    if trace:
        _CACHE["last_results"] = res
        if res.exec_time_ns is not None:
            print(f"HW exec time: {res.exec_time_ns} ns")

    out = np.zeros((B, T, C), dtype=np.float32)
    for core in range(NCORES):
        out[core // 4] += res.results[core]["out"]
    return out


# revision 7
# speedup vs baseline: 1.3178x; 1.0586x over previous
"""Causal self-attention (B=2, T=2048, C=2048, H=16, D=128) on 8 trn2 cores.

Sharding: tensor-parallel over heads x data-parallel over batch.
Core c handles batch c//4, heads [4*(c%4) .. 4*(c%4)+4). Each core computes
qkv projection for its 4 heads, RoPE, causal attention, and a partial
output projection (its heads' rows of W_proj); the host sums the 4 partials
per batch.

v3 design (all matmuls bf16 -> PSUM f32; ~4e-3 max-rel error):
  - Q^T/K^T/V live in SBUF in bf16 for the whole kernel: no DRAM scratch.
  - Weights are repacked host-side to partition-major so every DMA moves
    contiguous 4KB runs per partition.
  - Phase 1: QKV projection per 512-col t-block; Q,K produced transposed
    (W^T x^T) with RoPE fused into the PSUM evacuation on DVE; V natural
    via x-as-stationary, evacuated to SBUF by ACT.
  - Phase 2 per (q-block, head), S^T orientation, software-pipelined
    depth 3 (S matmuls run ahead of dn/pv so the ACT exp latency is
    hidden). Diagonal tiles only compute the valid column range
    (q_rel >= j*128); causal masking is a multiplicative 0/1 triangular
    [128,128] bf16 mask on DVE applied post-exp to the one mixed tile.
    denominators: ones^T @ P^T on PE; O^T = PV * recip(dn).
  - Phase 3 interleaved per q-block; PSUM evacuated by DVE (ACT keeps
    doing exp; DMA can't read PSUM); out f32 partials summed on host.
  - PSUM banks: st ring 4 (shared with phase-3 po tiles) + pv 2 + dn 2.
"""

import math
import os

import numpy as np

B, T, C = 2, 2048, 2048
H, D = 16, 128
HPC = 4  # heads per core
NCORES = 8

_CACHE = {}


def _build_program():
    import concourse.tile as tile
    from concourse import bacc, mybir

    f32 = mybir.dt.float32
    bf16 = mybir.dt.bfloat16
    Exp = mybir.ActivationFunctionType.Exp
    SCALE = 1.0 / math.sqrt(float(D))

    nc = bacc.Bacc(
        "TRN2", target_bir_lowering=False, debug=False, num_devices=NCORES
    )

    KT = C // 128  # 16 contraction tiles
    NTB = T // 512  # 4 t-blocks
    MORD = (0, 4, 1, 5, 2, 6, 3, 7)

    # Partition-major packed layouts (see make_in_maps).
    xP = nc.dram_tensor("xP", [128, NTB, KT, 512], bf16, kind="ExternalInput").ap()
    wqkP = nc.dram_tensor("wqkP", [128, 8 * KT * 128], bf16, kind="ExternalInput").ap()
    wvP = nc.dram_tensor("wvP", [128, KT * 512], bf16, kind="ExternalInput").ap()
    wp = nc.dram_tensor("wp", [HPC * D, C], bf16, kind="ExternalInput").ap()
    onesr = nc.dram_tensor("onesr", [128, 128], bf16, kind="ExternalInput").ap()
    cosT = nc.dram_tensor("cosT", [128, T], f32, kind="ExternalInput").ap()
    sinTs = nc.dram_tensor("sinTs", [128, T], f32, kind="ExternalInput").ap()
    mask01 = nc.dram_tensor("mask01", [128, 128], bf16, kind="ExternalInput").ap()
    out = nc.dram_tensor("out", [T, C], bf16, kind="ExternalOutput").ap()

    with tile.TileContext(nc) as tc:
        with (
            tc.tile_pool(name="consts", bufs=1) as consts,
            tc.tile_pool(name="pers", bufs=1) as pers,
        ):
            ones_sb = consts.tile([128, 128], bf16, tag="ones")
            m01_sb = consts.tile([128, 128], bf16, tag="m01")
            cos_sb = consts.tile([128, T], f32, tag="cos")
            sin_sb = consts.tile([128, T], f32, tag="sin")

            qt = [
                pers.tile([128, T], bf16, tag=f"qt{h}", name=f"qt{h}")
                for h in range(HPC)
            ]
            kt = [
                pers.tile([128, T], bf16, tag=f"kt{h}", name=f"kt{h}")
                for h in range(HPC)
            ]
            vt = pers.tile([128, KT, 512], bf16, tag="vt")
            o2 = [
                pers.tile([128, T], bf16, tag=f"o2_{h}", name=f"o2_{h}")
                for h in range(HPC)
            ]
            wps = [
                pers.tile([128, T], bf16, tag=f"wp{i}", name=f"wp{i}")
                for i in range(HPC)
            ]

            # ---------------- Phase 1: QKV projection ----------------
            with (
                tc.tile_pool(name="p1x", bufs=2) as p1x,
                tc.tile_pool(name="p1w", bufs=1) as p1w,
                tc.tile_pool(name="p1e", bufs=2) as p1e,
                tc.tile_pool(name="p1qk", bufs=3, space="PSUM") as p1qk,
                tc.tile_pool(name="p1v", bufs=2, space="PSUM") as p1v,
            ):
                wqkg = p1w.tile([128, 8, KT, 128], bf16, tag="wqkg")
                wv_sb = p1w.tile([128, KT, 512], bf16, tag="wv")

                def load_wm(m):
                    nc.sync.dma_start(
                        out=wqkg[:, m], in_=wqkP[:, m * 2048 : (m + 1) * 2048]
                    )

                def load_x(xtile, tb):
                    for kg in range(4):
                        nc.sync.dma_start(
                            out=xtile[:, kg * 4 : (kg + 1) * 4, :],
                            in_=xP[:, tb, kg * 4 : (kg + 1) * 4, :],
                        )

                # Preamble: what the first m-chain needs goes first; cos/sin
                # are only needed by the (DVE) evacuation, which trails PE.
                nc.sync.dma_start(out=ones_sb, in_=onesr)
                nc.sync.dma_start(out=m01_sb, in_=mask01)
                load_wm(MORD[0])
                xtb0 = p1x.tile([128, KT, 512], bf16, tag="xtb")
                load_x(xtb0, 0)
                nc.sync.dma_start(out=cos_sb[:, 0:512], in_=cosT[:, 0:512])
                nc.sync.dma_start(out=sin_sb[:, 0:512], in_=sinTs[:, 0:512])
                for m in MORD[1:]:
                    load_wm(m)
                for kg in range(4):
                    nc.sync.dma_start(
                        out=wv_sb[:, kg * 4 : (kg + 1) * 4, :],
                        in_=wvP[:, kg * 2048 : (kg + 1) * 2048],
                    )
                xtb1 = p1x.tile([128, KT, 512], bf16, tag="xtb", name="xtb1")
                load_x(xtb1, 1)
                for tbb in range(1, NTB):
                    s = slice(tbb * 512, (tbb + 1) * 512)
                    nc.sync.dma_start(out=cos_sb[:, s], in_=cosT[:, s])
                    nc.sync.dma_start(out=sin_sb[:, s], in_=sinTs[:, s])
                for i in range(HPC):
                    nc.sync.dma_start(out=wps[i], in_=wp[i * 128 : (i + 1) * 128, :])

                xtbs = [xtb0, xtb1, None, None]
                for tb in range(NTB):
                    tsl = slice(tb * 512, (tb + 1) * 512)
                    if tb + 2 < NTB:
                        xn = p1x.tile([128, KT, 512], bf16, tag="xtb",
                                      name=f"xtb{tb + 2}")
                        load_x(xn, tb + 2)
                        xtbs[tb + 2] = xn
                    xtb = xtbs[tb]
                    for m in MORD:
                        ps = p1qk.tile([128, 512], f32, tag="qk")
                        for k in range(KT):
                            nc.tensor.matmul(
                                ps,
                                lhsT=wqkg[:, m, k, :],
                                rhs=xtb[:, k, :],
                                start=(k == 0),
                                stop=(k == KT - 1),
                            )
                        # RoPE fused with PSUM evacuation (DVE), bf16 out.
                        dst = qt[m][:, tsl] if m < 4 else kt[m - 4][:, tsl]
                        tmp = p1e.tile([128, 512], f32, tag="rtmp")
                        nc.vector.tensor_mul(
                            tmp[0:64], ps[64:128], sin_sb[0:64, tsl]
                        )
                        nc.vector.tensor_mul(
                            tmp[64:128], ps[0:64], sin_sb[64:128, tsl]
                        )
                        tmp2 = p1e.tile([128, 512], f32, tag="rtmp2")
                        nc.vector.tensor_mul(tmp2, ps, cos_sb[:, tsl])
                        nc.vector.tensor_add(dst, tmp2, tmp)
                    for tsub in range(4):
                        psv = p1v.tile([128, 512], f32, tag="v")
                        for k in range(KT):
                            nc.tensor.matmul(
                                psv,
                                lhsT=xtb[:, k, tsub * 128 : (tsub + 1) * 128],
                                rhs=wv_sb[:, k, :],
                                start=(k == 0),
                                stop=(k == KT - 1),
                            )
                        nc.scalar.copy(vt[:, tb * 4 + tsub, :], psv)

            # ------------- Phases 2+3 fused per q-block -------------
            with (
                tc.tile_pool(name="p2ps", bufs=1, space="PSUM") as p2ps,
                tc.tile_pool(name="p2pt", bufs=5) as p2pt,
                tc.tile_pool(name="p2s", bufs=2) as p2s,
            ):
                DEPTH = 3
                pending = {}

                def emit_S(qb, h, kb):
                    j = kb - 4 * qb  # >=0 on the diagonal group
                    off = j * 128 if j > 0 else 0
                    st = p2ps.tile([128, 512], f32, tag="st", bufs=4)
                    nc.tensor.matmul(
                        st[:, off:],
                        lhsT=kt[h][:, kb * 128 : (kb + 1) * 128],
                        rhs=qt[h][:, qb * 512 + off : (qb + 1) * 512],
                        start=True,
                        stop=True,
                    )
                    pt = p2pt.tile([128, 512], bf16, tag="pt", bufs=5)
                    nc.scalar.activation(pt[:, off:], st[:, off:], Exp, scale=SCALE)
                    if j >= 0:
                        nc.vector.tensor_mul(
                            pt[:, off : off + 128], pt[:, off : off + 128], m01_sb
                        )
                    pending[(qb, h, kb)] = (pt, off)

                for qb in range(NTB):
                    qsl = slice(qb * 512, (qb + 1) * 512)
                    nk = 4 * (qb + 1)
                    for h in range(HPC):
                        pv = p2ps.tile([128, 512], f32, tag="pv", bufs=2)
                        dn = p2ps.tile([128, 512], f32, tag="dn", bufs=2)
                        for kb in range(min(DEPTH, nk)):
                            if (qb, h, kb) not in pending:
                                emit_S(qb, h, kb)
                        for kb in range(nk):
                            if kb + DEPTH < nk:
                                emit_S(qb, h, kb + DEPTH)
                            pt, off = pending.pop((qb, h, kb))
                            nc.tensor.matmul(
                                dn[:, off:],
                                lhsT=ones_sb,
                                rhs=pt[:, off:],
                                start=(kb == 0),
                                stop=(kb == nk - 1),
                            )
                            nc.tensor.matmul(
                                pv[:, off:],
                                lhsT=vt[:, kb, h * 128 : (h + 1) * 128],
                                rhs=pt[:, off:],
                                start=(kb == 0),
                                stop=(kb == nk - 1),
                            )
                        # dn holds the denominator on every partition.
                        rb = p2s.tile([128, 512], f32, tag="rb")
                        nc.vector.reciprocal_approx_fast(out=rb, in_=dn)
                        nc.vector.tensor_mul(o2[h][:, qsl], pv, rb)
                    # Prime next q-block's first head so its exps overlap
                    # this q-block's projection matmuls below.
                    if qb + 1 < NTB:
                        for kb in range(DEPTH):
                            emit_S(qb + 1, 0, kb)
                    # Phase 3 for this q-block's four 128-row t-tiles.
                    for tt in range(4):
                        t = qb * 4 + tt
                        tsl = slice(t * 128, (t + 1) * 128)
                        for half in range(2):
                            pos = [
                                p2ps.tile([128, 512], f32, tag="st", bufs=4,
                                          name=f"po{t}_{half}_{i}")
                                for i in range(2)
                            ]
                            for hd in range(HPC):
                                for i in range(2):
                                    cb = half * 2 + i
                                    nc.tensor.matmul(
                                        pos[i],
                                        lhsT=o2[hd][:, tsl],
                                        rhs=wps[hd][:, cb * 512 : (cb + 1) * 512],
                                        start=(hd == 0),
                                        stop=(hd == HPC - 1),
                                    )
                            for i in range(2):
                                cb = half * 2 + i
                                ob = p2s.tile([128, 512], bf16, tag="ob", bufs=4)
                                # Alternate evacuation engine so neither ACT
                                # nor DVE gates the phase-3 PSUM ring.
                                if i == 0:
                                    nc.scalar.copy(ob, pos[i])
                                else:
                                    nc.vector.tensor_copy(ob, pos[i])
                                nc.sync.dma_start(
                                    out=out[tsl, cb * 512 : (cb + 1) * 512],
                                    in_=ob,
                                )
    nc.compile()
    return nc


def _get_program():
    if "nc" not in _CACHE:
        _CACHE["nc"] = _build_program()
    return _CACHE["nc"]


def make_in_maps(x, cos, sin, W_qkv, W_proj):
    """Host-side sharding: per-core input dicts (bf16, partition-major)."""
    import ml_dtypes

    bf = ml_dtypes.bfloat16
    x = np.asarray(x, dtype=np.float32)
    cos = np.asarray(cos, dtype=np.float32)
    sin = np.asarray(sin, dtype=np.float32)
    W_qkv = np.asarray(W_qkv, dtype=np.float32)
    W_proj = np.asarray(W_proj, dtype=np.float32)

    cosT = np.ascontiguousarray(np.tile(cos.T, (2, 1)))  # [128, T]
    sinT = np.ascontiguousarray(np.concatenate([-sin.T, sin.T], axis=0))
    onesr = np.ones((128, 128), dtype=bf)
    # Valid (unmasked) iff q-col >= k-partition within the mixed tile.
    mask01 = np.triu(np.ones((128, 128), dtype=np.float32)).astype(bf)

    in_maps = []
    for core in range(NCORES):
        b, hg = core // 4, core % 4
        csl = slice(hg * 512, (hg + 1) * 512)
        wqk_np = np.concatenate(
            [W_qkv[:, csl], W_qkv[:, C + hg * 512 : C + (hg + 1) * 512]],
            axis=1,
        ).astype(bf)  # [C, 1024]
        # -> [128, (m, k, 128)] partition-major
        wqkP = np.ascontiguousarray(
            wqk_np.reshape(16, 128, 8, 128).transpose(1, 2, 0, 3).reshape(128, -1)
        )
        wv_np = W_qkv[:, 2 * C + hg * 512 : 2 * C + (hg + 1) * 512].astype(bf)
        wvP = np.ascontiguousarray(
            wv_np.reshape(16, 128, 512).transpose(1, 0, 2).reshape(128, -1)
        )
        wp_np = np.ascontiguousarray(W_proj[hg * 512 : (hg + 1) * 512, :]).astype(bf)
        # x[b].T [C, T] -> [128, (tb, k, 512)] partition-major
        xT_np = x[b].T.astype(bf)  # [C, T]
        xPm = np.ascontiguousarray(
            xT_np.reshape(16, 128, 4, 512).transpose(1, 2, 0, 3)
        )  # [128, 4, 16, 512]
        in_maps.append(
            {
                "xP": xPm,
                "wqkP": wqkP,
                "wvP": wvP,
                "wp": wp_np,
                "onesr": onesr,
                "cosT": cosT,
                "sinTs": sinT,
                "mask01": mask01,
            }
        )
    return in_maps


def kernel(x, cos, sin, W_qkv, W_proj):
    from concourse.bass_utils import run_bass_kernel_spmd

    nc = _get_program()
    in_maps = make_in_maps(x, cos, sin, W_qkv, W_proj)
    trace = bool(int(os.environ.get("KERNEL_TRACE", "0")))
    res = run_bass_kernel_spmd(
        nc, in_maps, core_ids=list(range(NCORES)), trace=trace
    )
    if trace:
        _CACHE["last_results"] = res
        if res.exec_time_ns is not None:
            print(f"HW exec time: {res.exec_time_ns} ns")

    out = np.zeros((B, T, C), dtype=np.float32)
    for core in range(NCORES):
        out[core // 4] += np.asarray(res.results[core]["out"], dtype=np.float32)
    return out
